# revision 4
# baseline (speedup 1.0000x reference)
"""Trainium2 Bass kernel for GQA sliding-window attention with RoPE.

Model (full problem):
  x [4096, 4096] -> q/k/v projections -> RoPE(q,k) -> GQA sliding-window
  attention (B=2 packed seqs of S=2048, window=1024) -> out proj [4096, 4096].

Sharding over 8 NeuronCores: tensor-parallel over 4 head-groups (8 q-heads /
2 kv-heads per group) x data-parallel over the 2 packed sequences.
core = g*2 + s.  Each core computes a partial out^T [4096, 2048] (its head
group's contribution for its sequence); the host sums the 4 group-partials
per sequence and transposes.

On-core dataflow (feature-major "transposed" activations throughout):
  phase 1: x^T tiles via PE transpose; q^T/k^T = W^T x^T with fused RoPE on
           PSUM eviction; v token-major.
  phase 2: per (q-tile, head): S = q^T.T k^T chunks (only the <=9 key tiles
           inside the causal sliding window), additive mask on the two edge
           tiles, exp on ACT (row-sums for free), scale by 1/l, PE-transpose
           P, PV accumulated over key tiles -> attn^T.
  phase 3: out^T = wo^T attn^T, streamed to DRAM.

All matmuls run as float32r (full-rate fp32 mode, ~1e-4 rounding).
"""

import sys

for _p in ("/opt/trn_rl_repo",):
    if _p not in sys.path:
        sys.path.insert(0, _p)

import numpy as np
import ml_dtypes

import concourse.bass as bass  # noqa: E402
import concourse.mybir as mybir  # noqa: E402
import concourse.tile as tile  # noqa: E402
from concourse import bacc  # noqa: E402
from concourse.bass_utils import run_bass_kernel_spmd  # noqa: E402

F32 = mybir.dt.float32
F32R = mybir.dt.float32r
BF16 = mybir.dt.bfloat16
AF = mybir.ActivationFunctionType
OP = mybir.AluOpType
from concourse import bass_isa  # noqa: E402
RED_ADD = bass_isa.ReduceOp.add

DIM = 4096
H = 32
KV = 8
HD = 128
B = 2
S = 2048
WINDOW = 1024
NEG = -100.0  # additive mask; exp(-100+s) == 0 to fp32 precision for |s|<~30

G = 4            # tensor-parallel head groups
HQ = H // G      # q heads per core = 8
HKV = KV // G    # kv heads per core = 2
N_CORES = 8

TOK = S          # tokens per core
CHUNK = 1024     # phase-1 token chunk
N_CHUNK = TOK // CHUNK
DT = DIM // 128  # 32 dim tiles
QT = TOK // 128  # 16 query tiles
W_KT = WINDOW // 128  # 8

BFNP = ml_dtypes.bfloat16
_NC = None


def _build(bench_iters=1, debug=False, phases="123"):
    nc = bacc.Bacc(None, target_bir_lowering=False)

    xT_d = nc.dram_tensor("xT", [DIM, TOK], BF16, kind="ExternalInput")
    wq_d = nc.dram_tensor("wq", [HQ, 4, 128, 8, 128], BF16, kind="ExternalInput")
    wk_d = nc.dram_tensor("wk", [HKV, 4, 128, 8, 128], BF16, kind="ExternalInput")
    wv_d = nc.dram_tensor("wv", [8, 128, 4, HKV * 128], BF16, kind="ExternalInput")
    wo_d = nc.dram_tensor("wo", [32, 128, HQ, 128], BF16, kind="ExternalInput")
    cos_d = nc.dram_tensor("cosT", [64, TOK], F32, kind="ExternalInput")
    sin_d = nc.dram_tensor("sinT", [64, TOK], F32, kind="ExternalInput")
    out_d = nc.dram_tensor("outT", [DIM, TOK], F32, kind="ExternalOutput")
    if debug:
        qT_dbg = nc.dram_tensor("qT_dbg", [128, HQ, TOK], F32, kind="ExternalOutput")
        kT_dbg = nc.dram_tensor("kT_dbg", [128, HKV, TOK], F32, kind="ExternalOutput")
        vS_dbg = nc.dram_tensor("vS_dbg", [128, QT, HKV * 128], F32, kind="ExternalOutput")
        at_dbg = nc.dram_tensor("at_dbg", [128, HQ, TOK], F32, kind="ExternalOutput")

    with tile.TileContext(nc) as tc:
        import contextlib
        loop_ctx = (tc.For_i(0, bench_iters, 1) if bench_iters > 1
                    else contextlib.nullcontext())
        with loop_ctx, tc.tile_pool(name="persist", bufs=1) as pp:
            qT = pp.tile([128, HQ, TOK], BF16, tag="qT")
            kT = pp.tile([128, HKV, TOK], BF16, tag="kT")
            vS = pp.tile([128, QT, HKV * 128], BF16, tag="vS")
            if phases != "123":
                # phase probes: keep every ExternalInput referenced so the
                # NEFF keeps the full parameter list
                tch = pp.tile([128, 16], BF16, tag="tch")
                nc.gpsimd.dma_start(tch[:, 0:2], xT_d[0:128, 0:2])
                nc.gpsimd.dma_start(tch[:, 2:4], wq_d[0, 0, :, 0, 0:2])
                nc.gpsimd.dma_start(tch[:, 4:6], wk_d[0, 0, :, 0, 0:2])
                nc.gpsimd.dma_start(tch[:, 6:8], wv_d[0, :, 0, 0:2])
                nc.gpsimd.dma_start(tch[:, 8:10], wo_d[0, :, 0, 0:2])
                tchf = pp.tile([128, 4], F32, tag="tchf")
                nc.gpsimd.dma_start(tchf[0:64, 0:2], cos_d[:, 0:2])
                nc.gpsimd.dma_start(tchf[0:64, 2:4], sin_d[:, 0:2])

            # ---------------- phase 1: QKV (+RoPE) -------------------------
            # x^T comes pre-transposed from the host.  Per 512-token chunk,
            # x^T tiles stream into a per-dim-tile ring; q/k features run in
            # groups of <=3 with dim-tile-outer loops so the next chunk's
            # x^T DMAs overlap the tail groups.  wk/wv stay resident.
            with tc.tile_pool(name="xTr", bufs=36) as xTr, \
                 tc.tile_pool(name="wvs", bufs=1) as wvs, \
                 tc.tile_pool(name="wqs", bufs=5) as wqs, \
                 tc.tile_pool(name="csp", bufs=1) as csp, \
                 tc.tile_pool(name="rtmp", bufs=3) as rt_p, \
                 tc.tile_pool(name="stp", bufs=3) as st_p, \
                 tc.tile_pool(name="ps_qk", bufs=3, space="PSUM") as ps_qk, \
                 tc.tile_pool(name="ps_v", bufs=2, space="PSUM") as ps_v:
                csb_c = csp.tile([64, TOK], F32, tag="csb_c")
                csb_s = csp.tile([64, TOK], F32, tag="csb_s")
                nc.gpsimd.dma_start(csb_c[:], cos_d[:])
                nc.gpsimd.dma_start(csb_s[:], sin_d[:])

                # wv stays resident (2MB bf16), loaded once
                wv_r = [wvs.tile([128, 4, HKV * 128], BF16, tag=f"wvr{d}",
                                 name=f"wvr{d}") for d in range(8)]
                for dtg in range(8):
                    nc.gpsimd.dma_start(wv_r[dtg][:], wv_d[dtg])

                GROUPS = [(0, 1, 2), (3, 4, 5), (6, 7, 8), (9,)]  # ft 8/9 = k0/k1

                def rope_evict(ps, ft, c):
                    if ft < HQ:
                        dst = qT[:, ft, c * CHUNK:(c + 1) * CHUNK]
                    else:
                        dst = kT[:, ft - HQ, c * CHUNK:(c + 1) * CHUNK]
                    cs_ = csb_c[:, c * CHUNK:(c + 1) * CHUNK]
                    sn_ = csb_s[:, c * CHUNK:(c + 1) * CHUNK]
                    # stage psum -> SBUF via ACT (idle in phase 1) so the
                    # psum tile frees fast; both halves land at base
                    # partition 0 (SB+SB tensor ops need equal bases)
                    st0 = st_p.tile([64, CHUNK], F32, tag="st0", name=f"st0_{c}_{ft}")
                    st1 = st_p.tile([64, CHUNK], F32, tag="st1", name=f"st1_{c}_{ft}")
                    nc.scalar.copy(st0[:], ps[0:64, :])
                    nc.scalar.copy(st1[:], ps[64:128, :])
                    t0c = rt_p.tile([64, CHUNK], F32, tag="rt", name=f"t0c_{c}_{ft}")
                    t1s = rt_p.tile([64, CHUNK], F32, tag="rt", name=f"t1s_{c}_{ft}")
                    t0s = rt_p.tile([64, CHUNK], F32, tag="rt", name=f"t0s_{c}_{ft}")
                    t1c = rt_p.tile([64, CHUNK], F32, tag="rt", name=f"t1c_{c}_{ft}")
                    nc.any.tensor_tensor(t0c[:], st0[:], cs_, OP.mult)
                    nc.any.tensor_tensor(t1s[:], st1[:], sn_, OP.mult)
                    nc.any.tensor_sub(dst[0:64, :], t0c[:], t1s[:])
                    nc.any.tensor_tensor(t0s[:], st0[:], sn_, OP.mult)
                    nc.any.tensor_tensor(t1c[:], st1[:], cs_, OP.mult)
                    nc.any.tensor_add(dst[64:128, :], t1c[:], t0s[:])

                for c in range(N_CHUNK if "1" in phases else 0):
                    xTt = []
                    for dt in range(DT):
                        t = xTr.tile([128, CHUNK], BF16, tag="xT",
                                     name=f"xT_{c}_{dt}")
                        nc.gpsimd.dma_start(
                            t[:], xT_d[dt * 128:dt * 128 + 128,
                                       c * CHUNK:(c + 1) * CHUNK])
                        xTt.append(t)
                    NH = CHUNK // 512  # psum-bank halves per chunk
                    for grp in GROUPS:
                        pss = {ft: ps_qk.tile([128, CHUNK], F32, tag="qk",
                                              name=f"qk_{c}_{ft}")
                               for ft in grp}
                        for dtg in range(4):
                            wts = {}
                            for ft in grp:
                                wt = wqs.tile([128, 8, 128], BF16, tag="w",
                                              name=f"w_{c}_{ft}_{dtg}")
                                src_ = (wq_d[ft, dtg] if ft < HQ
                                        else wk_d[ft - HQ, dtg])
                                (nc.sync if dtg % 2 == 0
                                 else nc.gpsimd).dma_start(wt[:], src_)
                                wts[ft] = wt
                            for j in range(8):
                                dt = dtg * 8 + j
                                for ft in grp:
                                    for hh in range(NH):
                                        nc.tensor.matmul(
                                            pss[ft][:, hh * 512:hh * 512 + 512],
                                            wts[ft][:, j, :],
                                            xTt[dt][:, hh * 512:hh * 512 + 512],
                                            start=(dtg == 0 and j == 0),
                                            stop=(dtg == 3 and j == 7))
                        for ft in grp:
                            rope_evict(pss[ft], ft, c)
                    # V (token-major): 2 psum tiles at a time, wv resident
                    for pas in range(CHUNK // 256):
                        psv = [ps_v.tile([128, HKV * 128], F32, tag="psv",
                                         name=f"psv_{c}_{pas}_{i}")
                               for i in range(2)]
                        for dt in range(DT):
                            for i in range(2):
                                t4 = pas * 2 + i
                                nc.tensor.matmul(
                                    psv[i],
                                    xTt[dt][:, t4 * 128:t4 * 128 + 128],
                                    wv_r[dt // 4][:, dt % 4, :],
                                    start=(dt == 0), stop=(dt == DT - 1))
                        for i in range(2):
                            nc.any.tensor_copy(
                                vS[:, c * (CHUNK // 128) + pas * 2 + i, :],
                                psv[i])

            # ---------------- phase 2: attention (S^T orientation) ----------
            # Per (head h, key-tile kt): S^T[k, q] for the q-window
            # [kt*128, (kt+9)*128) that kt participates in.  exp on ACT gives
            # P^T (bf16) directly; the causal-diagonal and window-far-edge
            # masks are applied by zeroing P^T triangles on GPSIMD after exp.
            # PV accumulates over kt into out^T psum per 512-col block qc,
            # with start=True on each kt's first-touch sub-range (no explicit
            # zero-init).  Row-sums l accumulate in bf16 SBUF via DVE
            # copy/add per piece; one partition_all_reduce per (h, qc) on
            # GPSIMD yields l broadcast across partitions.  Eviction divides
            # by l and writes attnT.
            with tc.tile_pool(name="attn", bufs=1) as attn_p:
                attnT = attn_p.tile([128, HQ, TOK], BF16, tag="attnT")
                if ("2" not in phases and "3" in phases) or \
                        ("1" not in phases and "2" in phases):
                    with tc.tile_pool(name="fillp", bufs=1) as fillp:
                        fz = fillp.tile([128, 2048], F32, tag="fz")
                        nc.vector.memset(fz[:], 0.0)
                        if "2" not in phases and "3" in phases:
                            for ft in range(HQ):
                                nc.vector.tensor_copy(attnT[:, ft, :], fz[:])
                        if "1" not in phases and "2" in phases:
                            for ft in range(HQ):
                                nc.vector.tensor_copy(qT[:, ft, :], fz[:])
                            for ft in range(HKV):
                                nc.vector.tensor_copy(kT[:, ft, :], fz[:])
                            for kt_ in range(QT):
                                nc.vector.tensor_copy(vS[:, kt_, :],
                                                      fz[:, 0:HKV * 128])
                with tc.tile_pool(name="PTk", bufs=5) as PTkp, \
                     tc.tile_pool(name="pacc", bufs=4) as paccp, \
                     tc.tile_pool(name="linv", bufs=4) as linvp, \
                     tc.tile_pool(name="ps_s1", bufs=2, space="PSUM") as ps_s1, \
                     tc.tile_pool(name="ps_s2", bufs=1, space="PSUM") as ps_s2, \
                     tc.tile_pool(name="ps_o", bufs=3, space="PSUM") as ps_o:
                    for h in range(HQ if "2" in phases else 0):
                        kvh = h // 4
                        outp = {}
                        lacc = {}
                        born = {}
                        pending = []

                        def emit_pv(job):
                            kt0, qlo0, bounds0, PTk0 = job
                            ft_b = 0 if kt0 == 0 else (kt0 + 8) * 128
                            for a, b in zip(bounds0[:-1], bounds0[1:]):
                                qc = a // 512
                                # start=True zeroes the whole 2KB psum bank,
                                # so only the first kt touching this qc may
                                # set it; later first-touch columns rely on
                                # the bank's pending-zero bytes.
                                first = a >= ft_b
                                last = (kt0 == min(QT - 1, 4 * qc + 3))
                                nc.tensor.matmul(
                                    outp[qc][:, a - qc * 512:b - qc * 512],
                                    vS[:, kt0, _kvh[0] * 128:_kvh[0] * 128 + 128],
                                    PTk0[:, a - qlo0:b - qlo0],
                                    start=(kt0 == born[qc]), stop=last,
                                    skip_group_check=True)
                                if first:
                                    nc.vector.tensor_copy(
                                        lacc[qc][:, a - qc * 512:b - qc * 512],
                                        PTk0[:, a - qlo0:b - qlo0])
                                else:
                                    nc.vector.tensor_tensor(
                                        lacc[qc][:, a - qc * 512:b - qc * 512],
                                        lacc[qc][:, a - qc * 512:b - qc * 512],
                                        PTk0[:, a - qlo0:b - qlo0], OP.add)
                            for qc in list(outp.keys()):
                                if kt0 == min(QT - 1, 4 * qc + 3):
                                    li = linvp.tile([128, 512], F32, tag="linv",
                                                    name=f"linv_{_h[0]}_{qc}")
                                    nc.gpsimd.partition_all_reduce(
                                        li[:], lacc[qc][:], 128, RED_ADD)
                                    nc.vector.reciprocal(li[:], li[:])
                                    nc.vector.tensor_tensor(
                                        attnT[:, _h[0], qc * 512:qc * 512 + 512],
                                        outp[qc][:], li[:], OP.mult)
                                    del outp[qc]
                                    del lacc[qc]

                        _h = [h]
                        _kvh = [kvh]
                        for kt in range(QT):
                            qlo, qhi = kt * 128, min((kt + 9) * 128, TOK)
                            # lazily allocate accumulators for newly covered qc
                            for qc in range((qlo // 512), (qhi + 511) // 512):
                                if qc not in outp:
                                    outp[qc] = ps_o.tile(
                                        [128, 512], F32, tag="outp",
                                        name=f"outp_{h}_{qc}")
                                    lacc[qc] = paccp.tile(
                                        [128, 512], BF16, tag="pacc",
                                        name=f"pacc_{h}_{qc}")
                                    born[qc] = kt
                            # drain one deferred PV job first so the PE
                            # queue holds ready PV work ahead of the S
                            # matmuls that wait on exp's psum-slot release
                            if len(pending) >= 2:
                                emit_pv(pending.pop(0))
                            ln = qhi - qlo
                            # PV/l pieces: 512-aligned (psum banks) + split at
                            # the first-touch boundary so start=True pieces
                            # exactly cover previously untouched psum columns
                            ft_b = 0 if kt == 0 else (kt + 8) * 128
                            bounds = set([qlo, qhi])
                            nb = (qlo // 512 + 1) * 512
                            while nb < qhi:
                                bounds.add(nb)
                                nb += 512
                            if qlo < ft_b < qhi:
                                bounds.add(ft_b)
                            bounds = sorted(bounds)
                            PTk = PTkp.tile([128, 1152], BF16, tag="PTk",
                                            name=f"PTk_{h}_{kt}")
                            # scores into a 2-bank psum tile (896 cols used
                            # when ln=1152, the 256-col tail in a 1-bank
                            # tile) so exp runs as 1-2 wide ACT calls
                            ln1 = 896 if ln == 1152 else min(ln, 1024)
                            Sp = ps_s1.tile([128, 1024], F32, tag="S1",
                                            name=f"S1_{h}_{kt}")
                            # matmul pieces must stay inside one psum bank
                            mp = [0]
                            while mp[-1] < ln1:
                                mp.append(min(mp[-1] + 512 - mp[-1] % 512, ln1))
                            for a, b in zip(mp[:-1], mp[1:]):
                                nc.tensor.matmul(
                                    Sp[:, a:b],
                                    kT[:, kvh, kt * 128:kt * 128 + 128],
                                    qT[:, h, qlo + a:qlo + b],
                                    start=True, stop=True)
                            nc.scalar.activation(
                                PTk[:, 0:ln1], Sp[:, 0:ln1], AF.Exp)
                            if ln1 < ln:
                                Sp2 = ps_s2.tile([128, 256], F32, tag="S2",
                                                 name=f"S2_{h}_{kt}")
                                nc.tensor.matmul(
                                    Sp2[:, :ln - ln1],
                                    kT[:, kvh, kt * 128:kt * 128 + 128],
                                    qT[:, h, qlo + ln1:qhi],
                                    start=True, stop=True)
                                nc.scalar.activation(
                                    PTk[:, ln1:ln], Sp2[:, :ln - ln1], AF.Exp)
                            # masks: zero the dead triangles of P^T in-place
                            # diag block (cols [0,128)): keep where q - k >= 0
                            nc.gpsimd.affine_select(
                                out=PTk[:, 0:128], in_=PTk[:, 0:128],
                                compare_op=OP.is_ge, fill=0.0,
                                base=0, pattern=[[1, 128]],
                                channel_multiplier=-1)
                            if kt + 8 < QT:  # far block: keep where k - q > 0
                                nc.gpsimd.affine_select(
                                    out=PTk[:, ln - 128:ln],
                                    in_=PTk[:, ln - 128:ln],
                                    compare_op=OP.is_ge, fill=0.0,
                                    base=-1, pattern=[[-1, 128]],
                                    channel_multiplier=1)
                            # PV + l accumulation deferred one kt so exp
                            # latency hides behind the next kt's scores
                            pending.append((kt, qlo, bounds, PTk))
                        while pending:
                            emit_pv(pending.pop(0))

                if debug:
                    nc.gpsimd.dma_start(qT_dbg[:], qT[:])
                    nc.gpsimd.dma_start(kT_dbg[:], kT[:])
                    with tc.tile_pool(name="dbgv", bufs=1) as dbgp:
                        vf = dbgp.tile([128, QT, HKV * 128], F32, tag="vf")
                        nc.vector.tensor_copy(vf[:], vS[:])
                        nc.gpsimd.dma_start(vS_dbg[:], vf[:])
                    nc.gpsimd.dma_start(at_dbg[:], attnT[:])

                # ---------------- phase 3: output projection ----------------
                with tc.tile_pool(name="wop", bufs=3) as wop, \
                     tc.tile_pool(name="outp", bufs=4) as outp, \
                     tc.tile_pool(name="ps_wo", bufs=2, space="PSUM") as ps_wo:
                    for do in range(32 if "3" in phases else 0):
                        wt = wop.tile([128, HQ, 128], BF16, tag="wo")
                        nc.sync.dma_start(wt[:], wo_d[do])
                        pso = ps_wo.tile([128, TOK], F32, tag="pso")
                        for ft in range(HQ):
                            for t4 in range(4):
                                nc.tensor.matmul(
                                    pso[:, t4 * 512:t4 * 512 + 512],
                                    wt[:, ft, :],
                                    attnT[:, ft, t4 * 512:t4 * 512 + 512],
                                    start=(ft == 0), stop=(ft == HQ - 1))
                        for t4 in range(4):
                            ob = outp.tile([128, 512], F32, tag="ob")
                            nc.scalar.copy(ob[:], pso[:, t4 * 512:t4 * 512 + 512])
                            nc.sync.dma_start(
                                out_d[do * 128:do * 128 + 128,
                                      t4 * 512:t4 * 512 + 512], ob[:])

    nc.compile()
    return nc


def _get_nc(bench_iters=1):
    global _NC
    if bench_iters != 1:
        return _build(bench_iters)
    if _NC is None:
        _NC = _build()
    return _NC


def _prep_inputs(x, cos, sin, wq, wk, wv, wo):
    """Shard + repack host-side.  Returns in_maps for cores g*2+s."""
    perm = np.concatenate([np.arange(0, HD, 2), np.arange(1, HD, 2)])
    scale = 1.0 / np.sqrt(np.float32(HD))
    # permute interleaved rope pairs to [evens; odds] per head; fold 1/sqrt(hd)
    wq_p = (wq.reshape(DIM, H, HD)[:, :, perm] * scale).astype(np.float32)
    wk_p = wk.reshape(DIM, KV, HD)[:, :, perm].astype(np.float32)
    wv_r = np.ascontiguousarray(wv.reshape(DIM, KV, HD))
    cosT = np.ascontiguousarray(cos[:S].T, dtype=np.float32)
    sinT = np.ascontiguousarray(sin[:S].T, dtype=np.float32)

    in_maps = []
    for g in range(G):
        # [dim, hq, hd] -> [ft, dtg, p, j, c]
        a = wq_p[:, g * HQ:(g + 1) * HQ, :].reshape(4, 8, 128, HQ, 128)
        wq_h = np.ascontiguousarray(a.transpose(3, 0, 2, 1, 4))
        a = wk_p[:, g * HKV:(g + 1) * HKV, :].reshape(4, 8, 128, HKV, 128)
        wk_h = np.ascontiguousarray(a.transpose(3, 0, 2, 1, 4))
        a = wv_r[:, g * HKV:(g + 1) * HKV, :].reshape(8, 4, 128, HKV * 128)
        wv_h = np.ascontiguousarray(a.transpose(0, 2, 1, 3))
        a = wo[g * HQ * HD:(g + 1) * HQ * HD, :].reshape(HQ, 128, 32, 128)
        wo_h = np.ascontiguousarray(a.transpose(2, 1, 0, 3))
        for s in range(B):
            xs = np.ascontiguousarray(x[s * S:(s + 1) * S].T).astype(BFNP)
            in_maps.append({
                "xT": xs, "wq": wq_h.astype(BFNP), "wk": wk_h.astype(BFNP),
                "wv": wv_h.astype(BFNP), "wo": wo_h.astype(BFNP),
                "cosT": cosT, "sinT": sinT,
            })
    return in_maps


def kernel(x, cos, sin, wq, wk, wv, wo, batch=B, window=WINDOW, **_):
    x = np.asarray(x)
    nc = _get_nc()
    in_maps = _prep_inputs(np.asarray(x, np.float32), np.asarray(cos, np.float32),
                           np.asarray(sin, np.float32), np.asarray(wq, np.float32),
                           np.asarray(wk, np.float32), np.asarray(wv, np.float32),
                           np.asarray(wo, np.float32))
    res = run_bass_kernel_spmd(nc, in_maps, core_ids=list(range(N_CORES)))
    out = np.zeros((B * S, DIM), np.float32)
    for g in range(G):
        for s in range(B):
            out[s * S:(s + 1) * S, :] += res.results[g * B + s]["outT"].T
    return out

